# revision 20
# baseline (speedup 1.0000x reference)
"""Trainium2 Bass kernel: embedding gather + Conv1d(k=5,pad=2) + positional add.

Sequence-parallel across 8 NeuronCores; core c computes output tokens
[c*1024, (c+1)*1024) for all 8 batch rows. b_lin/b_conv/pos_table fold into
one per-position bias table on the host.

Per core, per batch row:
  1. ONE indirect-DMA gather (1152 rows, offset ap [128, 9]) of bf16
     embedding rows with a zero pad row at table index 100000
     -> g[tok 128p, 1152f]  (g[p, j*128+d] = row of token j*128+p).
  2. ONE XBAR DMA transpose (HWDGE) g -> embT[din, 9, 128] (blockwise:
     embT[d, j, t] = g[t, j*128+d], i.e. [din, token] flat).
  3. conv as 5 accumulating bf16 matmuls per 512-token tile (k-outer over
     both tiles to share LDWEIGHTS):
     psum[dout, n] += W_k[din,dout]^T @ embT[din, n+k]
  4. DVE adds the folded bias ([dout, tok] f32) reading PSUM directly;
     HWDGE DMAs out_row[dout, 1024] f32 out. Host untransposes.
"""

import os
import sys

sys.path.insert(0, "/opt/trn_rl_repo")

import numpy as np

VOCAB = 100000
MAX_SEQ = 8192
DIM = 128
KW = 5
PAD = 2
B = 8
NCORES = 8
CHUNK = MAX_SEQ // NCORES
NBLK = 9
GATH = NBLK * 128

_CACHE = {}


def _build_nc(iters=1):
    from concourse import bacc, bass, mybir, tile

    f32 = mybir.dt.float32
    bf16 = mybir.dt.bfloat16
    i32 = mybir.dt.int32

    nc = bacc.Bacc(None, target_bir_lowering=False)
    table_d = nc.declare_dram_parameter("table", [VOCAB + 1, DIM], bf16, isOutput=False)
    idx_d = nc.declare_dram_parameter("idx", [128, B * NBLK], i32, isOutput=False)
    bias_d = nc.declare_dram_parameter("bias", [128, CHUNK], f32, isOutput=False)
    wk_d = nc.declare_dram_parameter("wk", [128, KW * DIM], bf16, isOutput=False)
    out_d = nc.declare_dram_parameter("out", [B, 128, CHUNK], f32, isOutput=True)

    with tile.TileContext(nc) as tc:
        with (
            tc.tile_pool(name="const", bufs=1) as constp,
            tc.tile_pool(name="g", bufs=4) as gpool,
            tc.tile_pool(name="embT", bufs=3) as epool,
            tc.tile_pool(name="orow", bufs=4) as orowp,
            tc.tile_pool(name="pc", bufs=4, space="PSUM") as cpool,
        ):
            idx_sb = constp.tile([128, B * NBLK], i32)
            nc.sync.dma_start(out=idx_sb[:, :], in_=idx_d[:, :])
            bias_sb = constp.tile([128, CHUNK], f32)
            nc.sync.dma_start(out=bias_sb[:, :], in_=bias_d[:, :])
            wk_sb = constp.tile([128, KW * DIM], bf16)
            nc.sync.dma_start(out=wk_sb[:, :], in_=wk_d[:, :])

            import contextlib

            loop_cm = (
                tc.For_i(0, iters, 1, hint_engines=(mybir.EngineType.PE,))
                if iters > 1
                else contextlib.nullcontext()
            )
            with loop_cm:
                body(nc, tc, bass, mybir, idx_sb, bias_sb, wk_sb,
                     table_d, out_d, gpool, epool, orowp, cpool)
    if not nc.is_finalized():
        nc.finalize()
    return nc


def body(nc, tc, bass, mybir, idx_sb, bias_sb, wk_sb, table_d, out_d,
         gpool, epool, orowp, cpool):
    f32 = mybir.dt.float32
    bf16 = mybir.dt.bfloat16
    # Batch rows per transpose group: few XBAR events (each stalls the SWDGE
    # gather stream ~5us) up front, small groups at the end to cut the tail.
    GROUPS = (4, 4)
    b0 = 0
    for grp, GB in enumerate(GROUPS):
        g = gpool.tile([128, GB * GATH], bf16, name=f"g{GB}")
        # The SWDGE ucode consumes ONE offset per partition (per contiguous
        # dest run), so each instruction gathers 128 rows into one 128-col
        # block. 9 instructions per batch row.
        for bl in range(GB):
            b = b0 + bl
            for blk in range(NBLK):
                col = b * NBLK + blk
                nc.gpsimd.indirect_dma_start(
                    out=g[:, (bl * NBLK + blk) * 128:(bl * NBLK + blk + 1) * 128],
                    out_offset=None,
                    in_=table_d[:, :],
                    in_offset=bass.IndirectOffsetOnAxis(
                        ap=idx_sb[:, col:col + 1], axis=0
                    ),
                )
        embT = epool.tile([128, GB * GATH], bf16, name=f"embT{GB}")
        teng = nc.sync if grp % 2 == 0 else nc.scalar
        teng.dma_start_transpose(
            out=embT[:, :].rearrange("p (a b) -> p a b", a=GB * NBLK),
            in_=g[:, :],
        )
        for bl in range(GB):
            b = b0 + bl
            out_row = orowp.tile([128, CHUNK], f32)
            pcs = [cpool.tile([128, 512], f32, name=f"pc{t}") for t in range(2)]
            for k in range(KW):
                for t in range(2):
                    nc.tensor.matmul(
                        out=pcs[t][:, :],
                        lhsT=wk_sb[:, k * DIM:(k + 1) * DIM],
                        rhs=embT[:, bl * GATH + t * 512 + k: bl * GATH + t * 512 + k + 512],
                        start=(k == 0),
                        stop=(k == KW - 1),
                    )
            for t in range(2):
                nc.vector.tensor_add(
                    out_row[:, t * 512:(t + 1) * 512],
                    pcs[t][:, :],
                    bias_sb[:, t * 512:(t + 1) * 512],
                )
            oeng = nc.scalar if b % 2 == 0 else nc.sync
            oeng.dma_start(out=out_d[b], in_=out_row[:, :])
        b0 += GB
    return nc


def _prep_inputs(X, W_lin, b_lin, W_conv, b_conv, pos_table):
    import ml_dtypes

    bf16 = ml_dtypes.bfloat16

    X = np.asarray(X)
    W_lin = np.asarray(W_lin, dtype=np.float32)
    b_lin = np.asarray(b_lin, dtype=np.float32)
    W_conv = np.asarray(W_conv, dtype=np.float32)
    b_conv = np.asarray(b_conv, dtype=np.float32)
    pos_table = np.asarray(pos_table, dtype=np.float32)

    table = np.empty((VOCAB + 1, DIM), dtype=bf16)
    table[:VOCAB] = W_lin.T.astype(bf16)
    table[VOCAB] = 0.0

    wb = np.einsum("oik,i->ko", W_conv, b_lin)
    conv_lin = np.broadcast_to(wb.sum(0), (MAX_SEQ, DIM)).copy()
    conv_lin[0] = wb[2:].sum(0)
    conv_lin[1] = wb[1:].sum(0)
    conv_lin[MAX_SEQ - 2] = wb[:4].sum(0)
    conv_lin[MAX_SEQ - 1] = wb[:3].sum(0)
    bias_total = conv_lin + b_conv[None, :] + pos_table

    wk_arr = np.ascontiguousarray(
        W_conv.transpose(1, 2, 0).reshape(DIM, KW * DIM)
    ).astype(bf16)

    Xi = X.astype(np.int64)
    j = np.arange(GATH)
    in_maps = []
    for c in range(NCORES):
        a = c * CHUNK + j - PAD
        valid = (a >= 0) & (a < MAX_SEQ)
        gi = np.where(valid[None, :], Xi[:, np.clip(a, 0, MAX_SEQ - 1)], VOCAB)
        idx_c = np.ascontiguousarray(
            gi.reshape(B, NBLK, 128).transpose(2, 0, 1).reshape(128, B * NBLK)
        ).astype(np.int32)
        bias_c = np.ascontiguousarray(bias_total[c * CHUNK:(c + 1) * CHUNK].T)
        in_maps.append({"table": table, "idx": idx_c, "bias": bias_c, "wk": wk_arr})
    return in_maps


def kernel(X, W_lin, b_lin, W_conv, b_conv, pos_table):
    from concourse.bass_utils import run_bass_kernel_spmd

    iters = int(os.environ.get("KERNEL_ITERS", "1"))
    key = ("nc", iters)
    if key not in _CACHE:
        _CACHE[key] = _build_nc(iters)
    nc = _CACHE[key]

    in_maps = _prep_inputs(X, W_lin, b_lin, W_conv, b_conv, pos_table)
    res = run_bass_kernel_spmd(nc, in_maps, core_ids=list(range(NCORES)))
    _CACHE["last_results"] = res

    full = np.empty((B, MAX_SEQ, DIM), dtype=np.float32)
    for c in range(NCORES):
        o = res.results[c]["out"]
        full[:, c * CHUNK:(c + 1) * CHUNK, :] = o.transpose(0, 2, 1)
    return full
